# revision 3
# baseline (speedup 1.0000x reference)
"""KSG mutual-information estimator (retrieval_knn) on 8 Trainium2 cores.

Row-parallel cdist+knn: each core owns 1024 of the 8192 rows.
Per 128-row tile:
  phase A: PE computes negated squared joint distances in strips
           (sq-norm rows folded into the matmul contraction), DVE/ACT
           evacuate + min-combine into a bf16 score slab, DVE max8 +
           max_index give the index of the 6th-smallest distance (kth).
  phase B: GPSIMD indirect-DMA gathers 2*X[kth] / 2*y[kth], PE forms
           anchor-difference dot products, and a fused compare+accumulate
           pass counts nx/ny per row.
Host finishes with the O(N) digamma/mean/relu scalar math (float64).

Self-contained: hardcodes shapes N=8192, dx=64, dy=16, K=5, 8 cores.
"""

import numpy as np
import ml_dtypes

N = 8192
DX = 64
DY = 16
K = 5
NCORES = 8
RPC = N // NCORES          # rows per core = 1024
P = 128                    # partitions
T = RPC // P               # row-tiles per core = 8
CHUNK = 512                # matmul free-dim chunk
NJC = N // CHUNK           # 16 column chunks

BF16 = ml_dtypes.bfloat16

_prog_cache = {}


def _build_program():
    """Build the SPMD Bass/Tile program (same NEFF runs on all 8 cores)."""
    import concourse.bacc as bacc
    import concourse.bass as bass
    import concourse.mybir as mybir
    import concourse.tile as tile
    from concourse.masks import make_identity

    fp32 = mybir.dt.float32
    bf16 = mybir.dt.bfloat16
    u32 = mybir.dt.uint32

    nc = bacc.Bacc(
        "TRN2",
        target_bir_lowering=False,
        debug=False,
        enable_asserts=False,
        num_devices=NCORES,
    )

    # DRAM I/O
    xta_d = nc.dram_tensor("xta", [DX + 2, N], bf16, kind="ExternalInput").ap()
    yta_d = nc.dram_tensor("yta", [DY + 2, N], bf16, kind="ExternalInput").ap()
    x2_d = nc.dram_tensor("x2", [N, DX], bf16, kind="ExternalInput").ap()
    y2_d = nc.dram_tensor("y2", [N, DY], bf16, kind="ExternalInput").ap()
    lx_d = nc.dram_tensor("lx", [DX + 2, RPC], bf16, kind="ExternalInput").ap()
    ly_d = nc.dram_tensor("ly", [DY + 2, RPC], bf16, kind="ExternalInput").ap()
    nsqx_d = nc.dram_tensor("nsqx", [P, T], fp32, kind="ExternalInput").ap()
    nsqy_d = nc.dram_tensor("nsqy", [P, T], fp32, kind="ExternalInput").ap()
    nx_d = nc.dram_tensor("nx", [P, T], fp32, kind="ExternalOutput").ap()
    ny_d = nc.dram_tensor("ny", [P, T], fp32, kind="ExternalOutput").ap()
    kidx_d = nc.dram_tensor("kidx", [P, T], u32, kind="ExternalOutput").ap()

    Alu = mybir.AluOpType
    Act = mybir.ActivationFunctionType

    with tile.TileContext(nc) as tc:
        with (
            tc.tile_pool(name="const", bufs=1) as cpool,
            tc.tile_pool(name="slab", bufs=2) as slab_pool,
            tc.tile_pool(name="sx", bufs=4) as sx_pool,
            tc.tile_pool(name="dm", bufs=2) as dm_pool,
            tc.tile_pool(name="sm", bufs=2) as sm_pool,
            tc.tile_pool(name="g", bufs=2) as g_pool,
            tc.tile_pool(name="px", bufs=2, space="PSUM") as px_pool,
            tc.tile_pool(name="py", bufs=2, space="PSUM") as py_pool,
            tc.tile_pool(name="wx", bufs=1, space="PSUM") as wx_pool,
            tc.tile_pool(name="wy", bufs=1, space="PSUM") as wy_pool,
            tc.tile_pool(name="tr", bufs=2, space="PSUM") as tr_pool,
        ):
            # ---- load constants ----
            xta_s = cpool.tile([DX + 2, N], bf16, tag="xta")
            nc.sync.dma_start(xta_s[:], xta_d[:])
            yta_s = cpool.tile([DY + 2, N], bf16, tag="yta")
            nc.sync.dma_start(yta_s[:], yta_d[:])
            lx_s = cpool.tile([DX + 2, RPC], bf16, tag="lx")
            nc.sync.dma_start(lx_s[:], lx_d[:])
            ly_s = cpool.tile([DY + 2, RPC], bf16, tag="ly")
            nc.sync.dma_start(ly_s[:], ly_d[:])
            nsqx_s = cpool.tile([P, T], fp32, tag="nsqx")
            nc.sync.dma_start(nsqx_s[:], nsqx_d[:])
            nsqy_s = cpool.tile([P, T], fp32, tag="nsqy")
            nc.sync.dma_start(nsqy_s[:], nsqy_d[:])
            ident = cpool.tile([P, P], bf16, tag="ident")
            make_identity(nc, ident[:])
            nx_acc = cpool.tile([P, T], fp32, tag="nxacc")
            ny_acc = cpool.tile([P, T], fp32, tag="nyacc")
            kidx_acc = cpool.tile([P, T], u32, tag="kidxacc")

            for t in range(T):
                rsl = slice(t * P, (t + 1) * P)

                # ---------- phase A: scores + top-k ----------
                slab = slab_pool.tile([P, N], bf16, tag="slab")
                for jc in range(NJC):
                    csl = slice(jc * CHUNK, (jc + 1) * CHUNK)
                    px = px_pool.tile([P, CHUNK], fp32, tag="px")
                    nc.tensor.matmul(
                        px[:], lhsT=lx_s[:, rsl], rhs=xta_s[:, csl],
                        start=True, stop=True,
                    )
                    py = py_pool.tile([P, CHUNK], fp32, tag="py")
                    nc.tensor.matmul(
                        py[:], lhsT=ly_s[:, rsl], rhs=yta_s[:, csl],
                        start=True, stop=True,
                    )
                    # sx = px - sqx_i   (ScalarE, per-partition bias)
                    sx = sx_pool.tile([P, CHUNK], bf16, tag="sx")
                    nc.scalar.activation(
                        sx[:], px[:], Act.Identity,
                        bias=nsqx_s[:, t : t + 1], scale=1.0,
                    )
                    # slab = min(py - sqy_i, sx)   (DVE fused)
                    nc.vector.scalar_tensor_tensor(
                        out=slab[:, csl], in0=py[:],
                        scalar=nsqy_s[:, t : t + 1], in1=sx[:],
                        op0=Alu.add, op1=Alu.min,
                    )

                top8 = sm_pool.tile([P, 8], bf16, tag="top8")
                nc.vector.max(out=top8[:], in_=slab[:])
                idx8 = sm_pool.tile([P, 8], u32, tag="idx8")
                nc.vector.max_index(idx8[:], top8[:], slab[:])
                nc.vector.tensor_copy(kidx_acc[:, t : t + 1], idx8[:, K : K + 1])

                # ---------- phase B: gather anchors + counts ----------
                xk2 = g_pool.tile([P, DX], bf16, tag="xk2")
                nc.gpsimd.indirect_dma_start(
                    out=xk2[:], out_offset=None, in_=x2_d[:],
                    in_offset=bass.IndirectOffsetOnAxis(
                        ap=idx8[:, K : K + 1], axis=0
                    ),
                )
                yk2 = g_pool.tile([P, DY], bf16, tag="yk2")
                nc.gpsimd.indirect_dma_start(
                    out=yk2[:], out_offset=None, in_=y2_d[:],
                    in_offset=bass.IndirectOffsetOnAxis(
                        ap=idx8[:, K : K + 1], axis=0
                    ),
                )

                # c = sqx_i - sqxk_i  (Square-accum gives 4*sqxk)
                sqs_x = g_pool.tile([P, DX], bf16, tag="sqsx")
                sq4x = sm_pool.tile([P, 1], fp32, tag="sq4x")
                nc.scalar.activation(
                    out=sqs_x[:], in_=xk2[:], func=Act.Square, accum_out=sq4x[:]
                )
                cx = sm_pool.tile([P, 1], fp32, tag="cx")
                nc.vector.tensor_scalar(
                    out=cx[:], in0=sq4x[:], scalar1=-0.25,
                    scalar2=nsqx_s[:, t : t + 1],
                    op0=Alu.mult, op1=Alu.subtract,
                )
                sqs_y = g_pool.tile([P, DY], bf16, tag="sqsy")
                sq4y = sm_pool.tile([P, 1], fp32, tag="sq4y")
                nc.scalar.activation(
                    out=sqs_y[:], in_=yk2[:], func=Act.Square, accum_out=sq4y[:]
                )
                cy = sm_pool.tile([P, 1], fp32, tag="cy")
                nc.vector.tensor_scalar(
                    out=cy[:], in0=sq4y[:], scalar1=-0.25,
                    scalar2=nsqy_s[:, t : t + 1],
                    op0=Alu.mult, op1=Alu.subtract,
                )

                # V.T = (2X_rows).T - (2Xk).T
                xkT = tr_pool.tile([DX, P], bf16, tag="tr")
                nc.tensor.transpose(xkT[:], xk2[:], ident[:])
                vxT = g_pool.tile([DX, P], bf16, tag="vxT")
                nc.vector.tensor_tensor(
                    out=vxT[:], in0=lx_s[0:DX, rsl], in1=xkT[:], op=Alu.subtract
                )
                ykT = tr_pool.tile([DY, P], bf16, tag="tr")
                nc.tensor.transpose(ykT[:], yk2[:], ident[:])
                vyT = g_pool.tile([DY, P], bf16, tag="vyT")
                nc.vector.tensor_tensor(
                    out=vyT[:], in0=ly_s[0:DY, rsl], in1=ykT[:], op=Alu.subtract
                )

                accx = sm_pool.tile([P, NJC], fp32, tag="accx")
                accy = sm_pool.tile([P, NJC], fp32, tag="accy")
                for jc in range(NJC):
                    csl = slice(jc * CHUNK, (jc + 1) * CHUNK)
                    wx = wx_pool.tile([P, CHUNK], fp32, tag="wx")
                    nc.tensor.matmul(
                        wx[:], lhsT=vxT[:], rhs=xta_s[0:DX, csl],
                        start=True, stop=True,
                    )
                    dmx = dm_pool.tile([P, CHUNK], bf16, tag="dmx")
                    nc.vector.tensor_scalar(
                        out=dmx[:], in0=wx[:], scalar1=cx[:],
                        scalar2=None, op0=Alu.is_ge, op1=Alu.add,
                        accum_out=accx[:, jc : jc + 1],
                    )
                    wy = wy_pool.tile([P, CHUNK], fp32, tag="wy")
                    nc.tensor.matmul(
                        wy[:], lhsT=vyT[:], rhs=yta_s[0:DY, csl],
                        start=True, stop=True,
                    )
                    dmy = dm_pool.tile([P, CHUNK], bf16, tag="dmy")
                    nc.vector.tensor_scalar(
                        out=dmy[:], in0=wy[:], scalar1=cy[:],
                        scalar2=None, op0=Alu.is_ge, op1=Alu.add,
                        accum_out=accy[:, jc : jc + 1],
                    )

                nc.vector.reduce_sum(
                    nx_acc[:, t : t + 1], accx[:], axis=mybir.AxisListType.X
                )
                nc.vector.reduce_sum(
                    ny_acc[:, t : t + 1], accy[:], axis=mybir.AxisListType.X
                )

            nc.sync.dma_start(nx_d[:], nx_acc[:])
            nc.sync.dma_start(ny_d[:], ny_acc[:])
            nc.sync.dma_start(kidx_d[:], kidx_acc[:])

    nc.compile()
    return nc


def get_program():
    if "nc" not in _prog_cache:
        _prog_cache["nc"] = _build_program()
    return _prog_cache["nc"]


def make_in_maps(X, y):
    """Host-side input prep: transposes, bf16 casts, split sq-norm rows."""
    X = np.asarray(X, np.float32)
    y = np.asarray(y, np.float32)
    sqx = np.sum(X * X, axis=1, dtype=np.float32)
    sqy = np.sum(y * y, axis=1, dtype=np.float32)

    def aug_t(A, sq, d):
        out = np.zeros((d + 2, N), dtype=BF16)
        out[0:d] = A.T.astype(BF16)
        hi = sq.astype(BF16)
        lo = (sq - hi.astype(np.float32)).astype(BF16)
        out[d] = hi
        out[d + 1] = lo
        return out

    xta = aug_t(X, sqx, DX)
    yta = aug_t(y, sqy, DY)
    x2 = (2.0 * X).astype(BF16)
    y2 = (2.0 * y).astype(BF16)

    in_maps = []
    for c in range(NCORES):
        rows = slice(c * RPC, (c + 1) * RPC)
        lx = np.full((DX + 2, RPC), -1.0, dtype=BF16)
        lx[0:DX] = x2[rows].T
        ly = np.full((DY + 2, RPC), -1.0, dtype=BF16)
        ly[0:DY] = y2[rows].T
        nsqx = (-sqx[rows]).reshape(T, P).T.copy()   # [p, t]
        nsqy = (-sqy[rows]).reshape(T, P).T.copy()
        in_maps.append(
            {
                "xta": xta, "yta": yta, "x2": x2, "y2": y2,
                "lx": lx, "ly": ly,
                "nsqx": np.ascontiguousarray(nsqx, np.float32),
                "nsqy": np.ascontiguousarray(nsqy, np.float32),
            }
        )
    return in_maps


def digamma(x):
    """Vectorized digamma for x >= 1 (float64, ~1e-12 accurate)."""
    x = np.asarray(x, np.float64).copy()
    res = np.zeros_like(x)
    for _ in range(6):
        m = x < 6.0
        if not m.any():
            break
        res[m] -= 1.0 / x[m]
        x[m] += 1.0
    inv = 1.0 / x
    inv2 = inv * inv
    res += (
        np.log(x) - 0.5 * inv
        - inv2 * (1.0 / 12.0 - inv2 * (1.0 / 120.0 - inv2 / 252.0))
    )
    return res


def finish(nx_all, ny_all):
    """Host scalar finish: mean digammas -> MI -> relu."""
    n_avg = np.mean(digamma(nx_all + 1.0) + digamma(ny_all + 1.0))
    mi = (
        digamma(np.float64(N)) + digamma(np.float64(K)) - 1.0 / K - n_avg
    ) / np.log(2.0)
    return np.float32(max(mi, 0.0))


def run_device(in_maps, trace=False):
    from concourse import bass_utils

    nc = get_program()
    res = bass_utils.run_bass_kernel_spmd(
        nc, in_maps, core_ids=list(range(NCORES)), trace=trace
    )
    return res


def _counts_from_results(results):
    nx_all = np.concatenate(
        [r["nx"].T.reshape(-1) for r in results]
    )  # [t,p] order -> global rows
    ny_all = np.concatenate([r["ny"].T.reshape(-1) for r in results])
    return nx_all, ny_all


def kernel(X, y):
    in_maps = make_in_maps(X, y)
    res = run_device(in_maps, trace=False)
    nx_all, ny_all = _counts_from_results(res.results)
    return finish(nx_all, ny_all)


# revision 12
# speedup vs baseline: 1.1579x; 1.1579x over previous
"""KSG mutual-information estimator (retrieval_knn) on 8 Trainium2 cores.

Row-parallel cdist+knn: each core owns 1024 of the 8192 rows.
Per 128-row tile:
  phase A: PE computes negated squared joint distances in strips
           (sq-norm rows folded into the matmul contraction), ScalarE
           evacuates both strips (per-partition -|x_i|^2 bias), GPSIMD
           min-combines them into a bf16 score slab, DVE max8 +
           max_index give the index of the 6th-smallest distance (kth).
  phase B: GPSIMD indirect-DMA gathers 2*X[kth] / 2*y[kth], PE forms
           anchor-difference dot products; ScalarE counts nx via a
           Sign+accumulate pass, DVE counts ny via is_ge+accumulate.
Host finishes with the O(N) digamma/mean/relu scalar math (float64).

Self-contained: hardcodes shapes N=8192, dx=64, dy=16, K=5, 8 cores.
"""

import numpy as np
import ml_dtypes

N = 8192
DX = 64
DY = 16
K = 5
NCORES = 8
RPC = N // NCORES          # rows per core = 1024
P = 128                    # partitions
T = RPC // P               # row-tiles per core = 8
CHUNK = 1024               # evac/count chunk (2 matmuls of 512 each)
NJC = N // CHUNK           # 8 column chunks
MMF = 512                  # matmul free dim (one PSUM bank)
SPLIT = 6                  # count chunks jc < SPLIT run on ScalarE (Sign),
                           # the rest on DVE (is_ge) -- engine balance knob

BF16 = ml_dtypes.bfloat16

_prog_cache = {}


def _build_program():
    """Build the SPMD Bass/Tile program (same NEFF runs on all 8 cores)."""
    import concourse.bacc as bacc
    import concourse.bass as bass
    import concourse.mybir as mybir
    import concourse.tile as tile
    from concourse.masks import make_identity

    fp32 = mybir.dt.float32
    bf16 = mybir.dt.bfloat16
    u32 = mybir.dt.uint32

    nc = bacc.Bacc(
        "TRN2",
        target_bir_lowering=False,
        debug=False,
        enable_asserts=False,
        num_devices=NCORES,
    )

    # DRAM I/O
    xta_d = nc.dram_tensor("xta", [DX + 2, N], bf16, kind="ExternalInput").ap()
    yta_d = nc.dram_tensor("yta", [DY + 2, N], bf16, kind="ExternalInput").ap()
    x2_d = nc.dram_tensor("x2", [N, DX], bf16, kind="ExternalInput").ap()
    y2_d = nc.dram_tensor("y2", [N, DY], bf16, kind="ExternalInput").ap()
    lx_d = nc.dram_tensor("lx", [DX + 2, RPC], bf16, kind="ExternalInput").ap()
    ly_d = nc.dram_tensor("ly", [DY + 2, RPC], bf16, kind="ExternalInput").ap()
    nsqx_d = nc.dram_tensor("nsqx", [P, T], fp32, kind="ExternalInput").ap()
    nsqy_d = nc.dram_tensor("nsqy", [P, T], fp32, kind="ExternalInput").ap()
    nx_d = nc.dram_tensor("nx", [P, T * NJC], fp32, kind="ExternalOutput").ap()
    ny_d = nc.dram_tensor("ny", [P, T * NJC], fp32, kind="ExternalOutput").ap()
    kidx_d = nc.dram_tensor("kidx", [P, T], u32, kind="ExternalOutput").ap()

    Alu = mybir.AluOpType
    Act = mybir.ActivationFunctionType

    with tile.TileContext(nc) as tc:
        with (
            tc.tile_pool(name="const", bufs=1) as cpool,
            tc.tile_pool(name="slab", bufs=2) as slab_pool,
            tc.tile_pool(name="ev", bufs=3) as ev_pool,
            tc.tile_pool(name="dm", bufs=2) as dm_pool,
            tc.tile_pool(name="sm", bufs=2) as sm_pool,
            tc.tile_pool(name="g", bufs=2) as g_pool,
            tc.tile_pool(name="px", bufs=1, space="PSUM") as px_pool,
            tc.tile_pool(name="py", bufs=1, space="PSUM") as py_pool,
            tc.tile_pool(name="wx", bufs=1, space="PSUM") as wx_pool,
            tc.tile_pool(name="wy", bufs=1, space="PSUM") as wy_pool,
        ):
            # ---- load constants ----
            xta_s = cpool.tile([DX + 2, N], bf16, tag="xta")
            nc.sync.dma_start(xta_s[:], xta_d[:])
            yta_s = cpool.tile([DY + 2, N], bf16, tag="yta")
            nc.sync.dma_start(yta_s[:], yta_d[:])
            lx_s = cpool.tile([DX + 2, RPC], bf16, tag="lx")
            nc.sync.dma_start(lx_s[:], lx_d[:])
            ly_s = cpool.tile([DY + 2, RPC], bf16, tag="ly")
            nc.sync.dma_start(ly_s[:], ly_d[:])
            nsqx_s = cpool.tile([P, T], fp32, tag="nsqx")
            nc.sync.dma_start(nsqx_s[:], nsqx_d[:])
            nsqy_s = cpool.tile([P, T], fp32, tag="nsqy")
            nc.sync.dma_start(nsqy_s[:], nsqy_d[:])
            ident = cpool.tile([P, P], bf16, tag="ident")
            make_identity(nc, ident[:])
            kidx_acc = cpool.tile([P, T], u32, tag="kidxacc")

            for t in range(T):
                rsl = slice(t * P, (t + 1) * P)
                nsx = nsqx_s[:, t : t + 1]
                nsy = nsqy_s[:, t : t + 1]

                # ---------- phase A: scores + top-k ----------
                slab = slab_pool.tile([P, N], bf16, tag="slab")
                for jc in range(NJC):
                    px = px_pool.tile([P, CHUNK], fp32, tag="px")
                    py = py_pool.tile([P, CHUNK], fp32, tag="py")
                    for h in range(CHUNK // MMF):
                        csl = slice(jc * CHUNK + h * MMF, jc * CHUNK + (h + 1) * MMF)
                        hsl = slice(h * MMF, (h + 1) * MMF)
                        nc.tensor.matmul(
                            px[:, hsl], lhsT=lx_s[:, rsl], rhs=xta_s[:, csl],
                            start=True, stop=True,
                        )
                        nc.tensor.matmul(
                            py[:, hsl], lhsT=ly_s[:, rsl], rhs=yta_s[:, csl],
                            start=True, stop=True,
                        )
                    # ScalarE: sx = px - sqx_i ; sy = py - sqy_i
                    sx = ev_pool.tile([P, CHUNK], bf16, tag="sx")
                    nc.scalar.activation(
                        sx[:], px[:], Act.Identity, bias=nsx, scale=1.0
                    )
                    sy = ev_pool.tile([P, CHUNK], bf16, tag="sy")
                    nc.scalar.activation(
                        sy[:], py[:], Act.Identity, bias=nsy, scale=1.0
                    )
                    # DVE: slab = min(sx, sy)  (bf16 SBUF 2x mode)
                    nc.vector.tensor_tensor(
                        out=slab[:, jc * CHUNK : (jc + 1) * CHUNK],
                        in0=sx[:], in1=sy[:], op=Alu.min,
                    )

                top8 = sm_pool.tile([P, 8], bf16, tag="top8")
                nc.vector.max(out=top8[:], in_=slab[:])
                idx8 = sm_pool.tile([P, 8], u32, tag="idx8")
                nc.vector.max_index(idx8[:], top8[:], slab[:])
                nc.vector.tensor_copy(kidx_acc[:, t : t + 1], idx8[:, K : K + 1])

                # ---------- phase B: gather anchors + counts ----------
                xk2 = g_pool.tile([P, DX], bf16, tag="xk2")
                nc.gpsimd.indirect_dma_start(
                    out=xk2[:], out_offset=None, in_=x2_d[:],
                    in_offset=bass.IndirectOffsetOnAxis(
                        ap=idx8[:, K : K + 1], axis=0
                    ),
                )
                yk2 = g_pool.tile([P, DY], bf16, tag="yk2")
                nc.gpsimd.indirect_dma_start(
                    out=yk2[:], out_offset=None, in_=y2_d[:],
                    in_offset=bass.IndirectOffsetOnAxis(
                        ap=idx8[:, K : K + 1], axis=0
                    ),
                )

                # Square+accum -> 4*|xk|^2 ; ncx = sqxk - sqx (x bias, Sign
                # count), cy = sqy - sqyk (y threshold, is_ge count)
                sqs_x = g_pool.tile([P, DX], bf16, tag="sqsx")
                sq4x = sm_pool.tile([P, 1], fp32, tag="sq4x")
                nc.scalar.activation(
                    out=sqs_x[:], in_=xk2[:], func=Act.Square, accum_out=sq4x[:]
                )
                ncx = sm_pool.tile([P, 1], fp32, tag="ncx")
                nc.vector.tensor_scalar(
                    out=ncx[:], in0=sq4x[:], scalar1=0.25, scalar2=nsx,
                    op0=Alu.mult, op1=Alu.add,
                )
                cx = sm_pool.tile([P, 1], fp32, tag="cx")
                nc.vector.tensor_scalar(
                    out=cx[:], in0=sq4x[:], scalar1=-0.25, scalar2=nsx,
                    op0=Alu.mult, op1=Alu.subtract,
                )
                sqs_y = g_pool.tile([P, DY], bf16, tag="sqsy")
                sq4y = sm_pool.tile([P, 1], fp32, tag="sq4y")
                nc.scalar.activation(
                    out=sqs_y[:], in_=yk2[:], func=Act.Square, accum_out=sq4y[:]
                )
                cy = sm_pool.tile([P, 1], fp32, tag="cy")
                nc.vector.tensor_scalar(
                    out=cy[:], in0=sq4y[:], scalar1=-0.25, scalar2=nsy,
                    op0=Alu.mult, op1=Alu.subtract,
                )
                ncy = sm_pool.tile([P, 1], fp32, tag="ncy")
                nc.vector.tensor_scalar(
                    out=ncy[:], in0=sq4y[:], scalar1=0.25, scalar2=nsy,
                    op0=Alu.mult, op1=Alu.add,
                )

                # V.T = (2X_rows).T - (2Xk).T  (PE transpose into count pools)
                xkT = wx_pool.tile([DX, P], bf16, tag="wx")
                nc.tensor.transpose(xkT[:], xk2[:], ident[:])
                vxT = g_pool.tile([DX, P], bf16, tag="vxT")
                nc.vector.tensor_tensor(
                    out=vxT[:], in0=lx_s[0:DX, rsl], in1=xkT[:], op=Alu.subtract
                )
                ykT = wy_pool.tile([DY, P], bf16, tag="wy")
                nc.tensor.transpose(ykT[:], yk2[:], ident[:])
                vyT = g_pool.tile([DY, P], bf16, tag="vyT")
                nc.vector.tensor_tensor(
                    out=vyT[:], in0=ly_s[0:DY, rsl], in1=ykT[:], op=Alu.subtract
                )

                accx = sm_pool.tile([P, NJC], fp32, tag="accx")
                accy = sm_pool.tile([P, NJC], fp32, tag="accy")
                for jc in range(NJC):
                    wx = wx_pool.tile([P, CHUNK], fp32, tag="wx")
                    wy = wy_pool.tile([P, CHUNK], fp32, tag="wy")
                    for h in range(CHUNK // MMF):
                        csl = slice(jc * CHUNK + h * MMF, jc * CHUNK + (h + 1) * MMF)
                        hsl = slice(h * MMF, (h + 1) * MMF)
                        nc.tensor.matmul(
                            wx[:, hsl], lhsT=vxT[:], rhs=xta_s[0:DX, csl],
                            start=True, stop=True,
                        )
                        nc.tensor.matmul(
                            wy[:, hsl], lhsT=vyT[:], rhs=yta_s[0:DY, csl],
                            start=True, stop=True,
                        )
                    if jc < SPLIT:
                        # ScalarE: acc_jc = sum_j sign(w - c)
                        dmx = dm_pool.tile([P, CHUNK], bf16, tag="dmx")
                        nc.scalar.activation(
                            out=dmx[:], in_=wx[:], func=Act.Sign, bias=ncx[:],
                            scale=1.0, accum_out=accx[:, jc : jc + 1],
                        )
                        dmy = dm_pool.tile([P, CHUNK], bf16, tag="dmy")
                        nc.scalar.activation(
                            out=dmy[:], in_=wy[:], func=Act.Sign, bias=ncy[:],
                            scale=1.0, accum_out=accy[:, jc : jc + 1],
                        )
                    else:
                        # DVE: acc_jc = #(w >= c)
                        dmx = dm_pool.tile([P, CHUNK], bf16, tag="dmx")
                        nc.vector.tensor_scalar(
                            out=dmx[:], in0=wx[:], scalar1=cx[:],
                            scalar2=None, op0=Alu.is_ge, op1=Alu.add,
                            accum_out=accx[:, jc : jc + 1],
                        )
                        dmy = dm_pool.tile([P, CHUNK], bf16, tag="dmy")
                        nc.vector.tensor_scalar(
                            out=dmy[:], in0=wy[:], scalar1=cy[:],
                            scalar2=None, op0=Alu.is_ge, op1=Alu.add,
                            accum_out=accy[:, jc : jc + 1],
                        )

                nc.sync.dma_start(nx_d[:, t * NJC : (t + 1) * NJC], accx[:])
                nc.sync.dma_start(ny_d[:, t * NJC : (t + 1) * NJC], accy[:])

            nc.sync.dma_start(kidx_d[:], kidx_acc[:])

    nc.compile()
    return nc


def get_program():
    if "nc" not in _prog_cache:
        _prog_cache["nc"] = _build_program()
    return _prog_cache["nc"]


def make_in_maps(X, y):
    """Host-side input prep: transposes, bf16 casts, split sq-norm rows."""
    X = np.asarray(X, np.float32)
    y = np.asarray(y, np.float32)
    sqx = np.sum(X * X, axis=1, dtype=np.float32)
    sqy = np.sum(y * y, axis=1, dtype=np.float32)

    def aug_t(A, sq, d):
        out = np.zeros((d + 2, N), dtype=BF16)
        out[0:d] = A.T.astype(BF16)
        hi = sq.astype(BF16)
        lo = (sq - hi.astype(np.float32)).astype(BF16)
        out[d] = hi
        out[d + 1] = lo
        return out

    xta = aug_t(X, sqx, DX)
    yta = aug_t(y, sqy, DY)
    x2 = (2.0 * X).astype(BF16)
    y2 = (2.0 * y).astype(BF16)

    in_maps = []
    for c in range(NCORES):
        rows = slice(c * RPC, (c + 1) * RPC)
        lx = np.full((DX + 2, RPC), -1.0, dtype=BF16)
        lx[0:DX] = x2[rows].T
        ly = np.full((DY + 2, RPC), -1.0, dtype=BF16)
        ly[0:DY] = y2[rows].T
        nsqx = (-sqx[rows]).reshape(T, P).T.copy()   # [p, t]
        nsqy = (-sqy[rows]).reshape(T, P).T.copy()
        in_maps.append(
            {
                "xta": xta, "yta": yta, "x2": x2, "y2": y2,
                "lx": lx, "ly": ly,
                "nsqx": np.ascontiguousarray(nsqx, np.float32),
                "nsqy": np.ascontiguousarray(nsqy, np.float32),
            }
        )
    return in_maps


def digamma(x):
    """Vectorized digamma for x >= 1 (float64, ~1e-12 accurate)."""
    x = np.asarray(x, np.float64).copy()
    res = np.zeros_like(x)
    for _ in range(6):
        m = x < 6.0
        if not m.any():
            break
        res[m] -= 1.0 / x[m]
        x[m] += 1.0
    inv = 1.0 / x
    inv2 = inv * inv
    res += (
        np.log(x) - 0.5 * inv
        - inv2 * (1.0 / 12.0 - inv2 * (1.0 / 120.0 - inv2 / 252.0))
    )
    return res


def finish(nx_all, ny_all):
    """Host scalar finish: mean digammas -> MI -> relu."""
    n_avg = np.mean(digamma(nx_all + 1.0) + digamma(ny_all + 1.0))
    mi = (
        digamma(np.float64(N)) + digamma(np.float64(K)) - 1.0 / K - n_avg
    ) / np.log(2.0)
    return np.float32(max(mi, 0.0))


def run_device(in_maps, trace=False):
    from concourse import bass_utils

    nc = get_program()
    res = bass_utils.run_bass_kernel_spmd(
        nc, in_maps, core_ids=list(range(NCORES)), trace=trace
    )
    return res


def _combine_chunks(raw):
    """raw [P, T*NJC]: per-chunk accumulators; chunks jc < SPLIT hold
    sum(sign(w-c)) (count = (S+CHUNK)/2), the rest hold direct counts."""
    arr = raw.reshape(P, T, NJC).astype(np.float64)
    arr = np.concatenate(
        [(arr[:, :, :SPLIT] + CHUNK) * 0.5, arr[:, :, SPLIT:]], axis=2
    )
    tot = arr.sum(axis=2)           # [P, T]
    return tot.T.reshape(-1)        # global row order t*P + p


def _counts_from_results(results):
    nx_all = np.concatenate([_combine_chunks(r["nx"]) for r in results])
    ny_all = np.concatenate([_combine_chunks(r["ny"]) for r in results])
    return nx_all, ny_all


def kernel(X, y):
    in_maps = make_in_maps(X, y)
    res = run_device(in_maps, trace=False)
    nx_all, ny_all = _counts_from_results(res.results)
    return finish(nx_all, ny_all)
